# revision 22
# baseline (speedup 1.0000x reference)
"""Multi-head causal self-attention (V=Q variant) on 8 Trainium2 cores.

Sharding: batch (2) x head-group (4 groups of 4 heads). Each core computes
full-sequence attention for its 4 heads of one batch element, plus its slice
of the output projection; the host sums the 4 partial projections per batch
and adds b0.

Per core: xT [1024, 2048], Wq_s/Wk_s [1024, 256], W0_s [256, 1024].

Scores are computed transposed (S^T[kv, q]) so the softmax denominator falls
out of the AV matmul via a ones-column appended to V (V aliases Q in this
module -- the reference replicates that bug). The 1/sqrt(DK) scale is folded
into Wk/bk on the host. All matmul inputs live in float32r SBUF tiles (TF32-
like, 1 cycle/row on the PE vs 4 for fp32); PSUM accumulation stays fp32.

ACT (exp) is the long-pole engine, so program order interleaves projection
sub-sweeps with attention j-blocks to start the exp stream as early as
possible. DVE does PSUM evictions + normalize, Pool does masks/broadcasts.
Causal diagonal tiles are narrowed to their valid q-range (left-trimmed).
"""

import ml_dtypes
import numpy as np

import concourse.bacc as bacc
import concourse.mybir as mybir
from concourse.tile import TileContext, add_dep_helper

P = 128
S = 2048  # sequence length
D = 1024  # model dim
HD = 256  # head-group width (4 heads x 64)
DK = 64
NQ = 4  # q chunks of 512
NKV = 16  # kv chunks of 128
NKD = 8  # D chunks of 128
F32 = mybir.dt.float32
F32R = mybir.dt.float32r
BF16 = mybir.dt.bfloat16
EXP = mybir.ActivationFunctionType.Exp

_CACHED_NC = None


def build_nc():
    nc = bacc.Bacc("TRN2", target_bir_lowering=False, debug=False, num_devices=8)
    xT = nc.declare_dram_parameter("xT", [D, S], BF16, isOutput=False)
    Wq = nc.declare_dram_parameter("Wq", [D, HD], BF16, isOutput=False)
    Wk = nc.declare_dram_parameter("Wk", [D, HD], BF16, isOutput=False)
    bqt = nc.declare_dram_parameter("bqt", [P, 2], F32, isOutput=False)
    bkt = nc.declare_dram_parameter("bkt", [P, 2], F32, isOutput=False)
    W0 = nc.declare_dram_parameter("W0", [HD, D], F32, isOutput=False)
    out = nc.declare_dram_parameter("out", [S, D], F32, isOutput=True)

    def rd(ap):
        # reinterpret a DRAM f32 region as f32r for raw DMA into f32r tiles
        return ap.bitcast(F32R)

    with TileContext(nc) as tc:
        with (
            tc.tile_pool(name="const", bufs=1) as const,
            tc.tile_pool(name="xt", bufs=16) as xtp,
            tc.tile_pool(name="wqk", bufs=1) as wp,
            tc.tile_pool(name="vp", bufs=32) as vpool,
            tc.tile_pool(name="pt", bufs=4) as ptp,
            tc.tile_pool(name="dp", bufs=4) as dpool,
            tc.tile_pool(name="ost", bufs=3) as ostp,
            tc.tile_pool(name="mm", bufs=2, space="PSUM") as mmp,
            tc.tile_pool(name="sps", bufs=2, space="PSUM") as spsum,
            tc.tile_pool(name="aps", bufs=2, space="PSUM") as apsum,
        ):
            identity = const.tile([P, P], F32)
            nc.gpsimd.memset(identity[:], 0.0)
            nc.gpsimd.affine_select(
                out=identity[:],
                in_=identity[:],
                compare_op=mybir.AluOpType.not_equal,
                fill=1.0,
                base=0,
                pattern=[[-1, P]],
                channel_multiplier=1,
            )
            ones_col = const.tile([P, 1], F32)
            nc.gpsimd.memset(ones_col[:], 1.0)
            # triangular mask [128,128]: keep (1.0) where q >= kv, i.e. f >= p
            tri = const.tile([P, P], F32, name="tri")
            nc.gpsimd.memset(tri[:], 1.0)
            nc.gpsimd.affine_select(
                out=tri[:],
                in_=tri[:],
                compare_op=mybir.AluOpType.is_ge,
                fill=0.0,
                base=0,
                pattern=[[1, P]],
                channel_multiplier=-1,
            )
            # [128,256] mask: zeros block then triangle (for left-padded tiles)
            ztri = const.tile([P, 256], F32, name="ztri")
            nc.gpsimd.memset(ztri[:], 1.0)
            nc.gpsimd.affine_select(
                out=ztri[:],
                in_=ztri[:],
                compare_op=mybir.AluOpType.is_ge,
                fill=0.0,
                base=-128,
                pattern=[[1, 256]],
                channel_multiplier=-1,
            )
            # ACT exp-table warmup while DMAs run
            warm = const.tile([P, 8], F32, name="warm")
            nc.gpsimd.memset(warm[:], 0.0)
            nc.scalar.activation(out=warm[:], in_=warm[:], func=EXP)

            bq_sb = const.tile([P, 2], F32)
            nc.sync.dma_start(out=bq_sb[:], in_=bqt[:, :])
            bk_sb = const.tile([P, 2], F32)
            nc.sync.dma_start(out=bk_sb[:], in_=bkt[:, :])
            w0_sb = [const.tile([P, D], F32R, name=f"w0_{kc}") for kc in range(2)]
            # QT/KT as [mi][ni] tiles of [128, 512] for fine-grained deps
            QT = [
                [const.tile([P, 512], F32R, name=f"qt{mi}_{ni}") for ni in range(NQ)]
                for mi in range(2)
            ]
            KT = [
                [const.tile([P, 512], F32R, name=f"kt{mi}_{ni}") for ni in range(NQ)]
                for mi in range(2)
            ]
            # normalized attention (transposed), per q-chunk and head-pair
            attn = [
                [const.tile([P, 512], F32R, name=f"attn{j}_{p}") for p in range(2)]
                for j in range(4)
            ]

            # weights in one strided DMA each (per-DMA issue cadence is ~650ns,
            # so many small DMAs would serialize the stream)
            wq_big = wp.tile([P, NKD, HD], BF16, name="wqb")
            nc.sync.dma_start(out=wq_big[:], in_=Wq.rearrange("(k p) c -> p k c", p=P))
            wk_big = wp.tile([P, NKD, HD], BF16, name="wkb")
            nc.sync.dma_start(out=wk_big[:], in_=Wk.rearrange("(k p) c -> p k c", p=P))
            wq_t = [wq_big[:, k, :] for k in range(NKD)]
            wk_t = [wk_big[:, k, :] for k in range(NKD)]
            # xT as [k][half] tiles of [128, 1024]; 4 serialized chains so
            # all half-0 tiles (cols 0:1024) land before any half-1.
            xh = [
                [xtp.tile([P, 1024], BF16, name="xtile") for _ in range(2)]
                for _ in range(NKD)
            ]
            # x half-0 chunks first, then half-1, then W0: the DMA path
            # drains in issue order, which staggers arrivals naturally
            for h in range(2):
                for k in range(NKD):
                    nc.sync.dma_start(
                        out=xh[k][h][:],
                        in_=xT[k * P : (k + 1) * P, h * 1024 : (h + 1) * 1024],
                    )
            for kc in range(2):
                nc.sync.dma_start(
                    out=w0_sb[kc][:], in_=rd(W0[kc * P : (kc + 1) * P, :])
                )

            def sweep_items(ni, mi):
                """Projection sub-sweep as a list of emit-thunks (per-k)."""
                half, col = divmod(ni, 2)
                pss = [mmp.tile([P, 512], F32, name="ps") for _ in range(2)]

                def mk(k):
                    def go():
                        for ps, wt in zip(pss, (wq_t, wk_t)):
                            nc.tensor.matmul(
                                ps[:],
                                lhsT=wt[k][:, mi * P : (mi + 1) * P],
                                rhs=xh[k][half][:, col * 512 : (col + 1) * 512],
                                start=(k == 0),
                                stop=(k == NKD - 1),
                            )
                    return go

                def evict():
                    for ps, bias, dstT in zip(pss, (bq_sb, bk_sb), (QT, KT)):
                        nc.vector.tensor_scalar_add(
                            dstT[mi][ni][:, :], ps[:], bias[:, mi : mi + 1]
                        )

                return [mk(k) for k in range(NKD)] + [evict]

            vp = {}

            def emit_transposes(pair, i_lo, i_hi):
                # V' tiles [128, 132]: A data 0:64, A one 64, B data 66:130, B one 130
                for i in range(i_lo, i_hi):
                    tp = spsum.tile([P, P], F32, name="spsA")
                    nc.tensor.transpose(
                        tp[:, 0:P],
                        QT[pair][i // 4][:, (i % 4) * P : (i % 4 + 1) * P].bitcast(F32),
                        identity[:],
                    )
                    vt = vpool.tile([P, 132], F32R, name="vt")
                    nc.vector.tensor_copy(vt[:, 0:64], tp[:, 0:64])
                    nc.vector.tensor_copy(vt[:, 66:130], tp[:, 64:128])
                    nc.gpsimd.tensor_copy(vt[:, 64:65], ones_col[:])
                    nc.gpsimd.tensor_copy(vt[:, 130:131], ones_col[:])
                    vp[(pair, i)] = vt

            bg = []  # drip queue of (cost, thunk): sweeps, then phase-C blocks

            def drip(budget):
                while bg and budget > 0:
                    cost, thunk = bg.pop(0)
                    thunk()
                    budget -= cost

            def emit_cblock_m(j, c):
                m = j * 4 + c
                ot = ostp.tile([P, D], F32, name="ot")
                for n in range(2):
                    ps = mmp.tile([P, 512], F32, name="ps")
                    for kc in range(2):
                        nc.tensor.matmul(
                            ps[:],
                            lhsT=attn[j][kc][:, c * P : (c + 1) * P],
                            rhs=w0_sb[kc][:, n * 512 : (n + 1) * 512],
                            start=(kc == 0),
                            stop=(kc == 1),
                        )
                    nc.vector.tensor_copy(ot[:, n * 512 : (n + 1) * 512], ps[:])
                    nc.sync.dma_start(
                        out=out[m * P : (m + 1) * P, n * 512 : (n + 1) * 512],
                        in_=ot[:, n * 512 : (n + 1) * 512],
                    )

            def emit_pair(pair):
                steps = [(j, i) for j in range(NQ) for i in range(4 * j + 4)]
                ats = {}
                pend = None

                def emit_S(j, i):
                    off = max(0, i * P - j * 512)  # 0,128,256,384
                    w = 512 - off
                    woff, wm = (256, 256) if w == P else (off, w)
                    kc = slice((i % 4) * P, (i % 4 + 1) * P)
                    qsl = slice(woff, woff + wm)
                    sA = spsum.tile([P, 512], F32, name="spsA")
                    sB = spsum.tile([P, 512], F32, name="spsB")
                    nc.tensor.matmul(
                        sA[:, 0:wm],
                        lhsT=KT[pair][i // 4][0:64, kc],
                        rhs=QT[pair][j][0:64, qsl],
                    )
                    nc.tensor.matmul(
                        sB[:, 0:wm],
                        lhsT=KT[pair][i // 4][64:128, kc],
                        rhs=QT[pair][j][64:128, qsl],
                    )
                    pA = ptp.tile([P, 512], F32R, name="ptA")
                    pB = ptp.tile([P, 512], F32R, name="ptB")
                    nc.scalar.activation(out=pA[:, 0:wm], in_=sA[:, 0:wm], func=EXP)
                    nc.scalar.activation(out=pB[:, 0:wm], in_=sB[:, 0:wm], func=EXP)
                    if i >= 4 * j:  # diagonal tile: mask leading block(s)
                        if wm > w:  # left-padded: zeros block + triangle
                            nc.vector.tensor_mul(pA[:, 0:256], pA[:, 0:256], ztri[:])
                            nc.vector.tensor_mul(pB[:, 0:256], pB[:, 0:256], ztri[:])
                        else:
                            nc.vector.tensor_mul(pA[:, 0:P], pA[:, 0:P], tri[:])
                            nc.vector.tensor_mul(pB[:, 0:P], pB[:, 0:P], tri[:])
                    return (j, i, pA, pB, qsl, wm)

                def emit_AV(j, i, pA, pB, qsl, wm):
                    if i == 0:
                        ats[j] = (
                            apsum.tile([P, 512], F32, name="aps"),
                            apsum.tile([P, 512], F32, name="aps"),
                        )
                    atA, atB = ats[j]
                    imax = 4 * j + 3
                    vt = vp[(pair, i)]
                    nc.tensor.matmul(
                        atA[0:65, qsl],
                        lhsT=vt[:, 0:65],
                        rhs=pA[:, 0:wm],
                        start=(i == 0),
                        stop=(i == imax),
                    )
                    nc.tensor.matmul(
                        atB[0:65, qsl],
                        lhsT=vt[:, 66:131],
                        rhs=pB[:, 0:wm],
                        start=(i == 0),
                        stop=(i == imax),
                    )
                    if i == imax:  # normalize: attn = att_un / d, d = row 64
                        for at, rows in ((atA, slice(0, 64)), (atB, slice(64, 128))):
                            rec = dpool.tile([1, 512], F32, name="rec")
                            nc.vector.reciprocal(rec[:], at[64:65, :])
                            rbc = dpool.tile([64, 512], F32, name="rbc")
                            nc.gpsimd.partition_broadcast(rbc[0:64, :], rec[0:1, :])
                            nc.vector.tensor_mul(
                                attn[j][pair][rows, :], at[0:64, :], rbc[0:64, :]
                            )
                        if pair == 1:  # output projection becomes available
                            for c in range(4):
                                bg.append((1, lambda j=j, c=c: emit_cblock_m(j, c)))

                for j, i in steps:
                    cur = emit_S(j, i)
                    drip(5)
                    if pend is not None:
                        emit_AV(*pend)
                    pend = cur
                emit_AV(*pend)

            # upfront: pair-0 ni=0 projection (DMA-paced) + first V transposes
            for it in sweep_items(0, 0):
                it()
            emit_transposes(0, 0, 4)
            # bg order follows need-by and DMA-arrival order; cost 2 paces the
            # first half-1-gated sweep to the chunk arrival rate
            def t_item(pair, i):
                return (1, lambda: emit_transposes(pair, i, i + 1))

            for ni, mi in ((1, 0),):
                bg.extend((1, it) for it in sweep_items(ni, mi))
            bg.extend(t_item(0, i) for i in range(4, 8))
            bg.extend((1, it) for it in sweep_items(2, 0))
            bg.extend(t_item(0, i) for i in range(8, 12))
            bg.extend((1, it) for it in sweep_items(3, 0))
            bg.extend(t_item(0, i) for i in range(12, 16))
            bg.extend((1, it) for it in sweep_items(0, 1))
            bg.extend((1, it) for it in sweep_items(1, 1))
            bg.extend(t_item(1, i) for i in range(0, 8))
            bg.extend((1, it) for it in sweep_items(2, 1))
            bg.extend((1, it) for it in sweep_items(3, 1))
            bg.extend(t_item(1, i) for i in range(8, 16))
            emit_pair(0)
            emit_pair(1)
            while bg:
                drip(1)

    nc.compile()
    return nc


def make_in_maps(pos_encode_toks, Wq, bq, Wk, bk, W0, b0):
    x = np.asarray(pos_encode_toks, dtype=np.float32)
    Wq = np.asarray(Wq, dtype=np.float32)
    bq = np.asarray(bq, dtype=np.float32)
    Wk = np.asarray(Wk, dtype=np.float32)
    bk = np.asarray(bk, dtype=np.float32)
    W0 = np.asarray(W0, dtype=np.float32)
    in_maps = []
    for core in range(8):
        b, g = divmod(core, 4)
        hs = slice(g * HD, (g + 1) * HD)
        scale = np.float32(1.0 / np.sqrt(DK))
        in_maps.append(
            {
                "xT": np.ascontiguousarray(x[b].T).astype(ml_dtypes.bfloat16),
                "Wq": np.ascontiguousarray(Wq[:, hs]).astype(ml_dtypes.bfloat16),
                "Wk": np.ascontiguousarray(Wk[:, hs] * scale).astype(ml_dtypes.bfloat16),
                "bqt": np.ascontiguousarray(bq[hs].reshape(2, P).T),
                "bkt": np.ascontiguousarray((bk[hs] * scale).reshape(2, P).T),
                "W0": np.ascontiguousarray(W0[hs, :]),
            }
        )
    return in_maps


def assemble(results, b0):
    out = np.zeros((2, S, D), dtype=np.float32)
    for core in range(8):
        b = core // 4
        out[b] += results[core]["out"]
    out += np.asarray(b0, dtype=np.float32)
    return out


def kernel(pos_encode_toks, Wq, bq, Wk, bk, W0, b0):
    from concourse.bass_utils import run_bass_kernel_spmd

    global _CACHED_NC
    if _CACHED_NC is None:
        _CACHED_NC = build_nc()
    in_maps = make_in_maps(pos_encode_toks, Wq, bq, Wk, bk, W0, b0)
    res = run_bass_kernel_spmd(_CACHED_NC, in_maps, core_ids=list(range(8)))
    return assemble(res.results, b0)
